# revision 14
# baseline (speedup 1.0000x reference)
"""Trainium2 Bass kernel: CapOnlyContrastiveLoss (margin contrastive loss, mean reduction).

reference math (N=8192, D=512, margin=0.2):
    scores[i,j]  = -||im_i - ex_j||        (via gemm identity)
    diag[i]      = -||im_i - s_i||         (only diag of l2_sim(im, s) is used)
    loss         = mean(relu(margin + scores - diag))

Strategy (v4): 4x2 core grid, fp8 DoubleRow MMs + row-tiled bf16 exsq-fold MMs,
grouped 4-bank epilogue.  vs v3: exsq scatter DMAs issued from the GpSimd DGE
(no SP head-of-line blocking), whole-quad DVE bf16 casts, fp8 casts on DVE,
im/dd stats via DVE bn_stats, and a 3-way epilogue split (DVE STT / DVE
tensor_scalar+accum / ACT relu+accum) to compare engine costs on HW.
"""

import numpy as np

import concourse.bacc as bacc
import concourse.bass as bass
import concourse.tile as tile
from concourse import bass_utils, mybir

N, D = 8192, 512
MARGIN = 0.2
P = 128
NJ = 512
GW = 2048
I_GROUPS, J_GROUPS = 4, 2
IM_R = N // I_GROUPS  # 2048
EX_R = N // J_GROUPS  # 4096
KC = D // P  # 4
N_IT = IM_R // P  # 16
N_JB = EX_R // GW  # 2
BANKS = GW // NJ  # 4
EX_PER_JB = GW // P  # 16
N_G = N_JB * N_IT  # 32

# per-group epilogue kind: 'stt' (DVE scalar_tensor_tensor min+acc),
# 'cache' (DVE tensor_scalar min+acc), 'act' (ACT relu(c-sq)+acc)
GROUP_KIND = ['stt'] * 16 + ['cache'] * 12 + ['act'] * 4

F32 = mybir.dt.float32
BF16 = mybir.dt.bfloat16
FP8 = mybir.dt.float8e4
AF = mybir.ActivationFunctionType
ALU = mybir.AluOpType
DR = mybir.MatmulPerfMode.DoubleRow

_CACHE = {}


def _emit(tc, nc, im_d, s_d, ex_d, acc_d, cvec_d):
    from contextlib import ExitStack

    with ExitStack() as ctx:
        singles = ctx.enter_context(tc.tile_pool(name="singles", bufs=1))
        exl = ctx.enter_context(tc.tile_pool(name="exl", bufs=3))
        iml = ctx.enter_context(tc.tile_pool(name="iml", bufs=3))
        casts = ctx.enter_context(tc.tile_pool(name="casts", bufs=3))
        tbuf = ctx.enter_context(tc.tile_pool(name="tbuf", bufs=3))
        scratch = ctx.enter_context(tc.tile_pool(name="scratch", bufs=6))
        sqp = ctx.enter_context(tc.tile_pool(name="sqp", bufs=2))
        psum = ctx.enter_context(tc.tile_pool(name="psum", bufs=2, space="PSUM"))
        dram = ctx.enter_context(tc.tile_pool(name="dram", bufs=1, space="DRAM"))

        imT8 = singles.tile([P, KC, IM_R], FP8)
        exT8s = [singles.tile([P, KC, GW], FP8, name=f"exT8_{j}") for j in range(N_JB)]
        imsq = singles.tile([P, N_IT], F32)
        cc = singles.tile([P, N_IT], F32)
        mv_im = singles.tile([P, N_IT, 2], F32)
        mv_dd = singles.tile([P, N_IT, 2], F32)
        exsq_cols = [singles.tile([P, EX_PER_JB], F32, name=f"exsqc{j}")
                     for j in range(N_JB)]
        acc_sb = singles.tile([P, N_G], F32)
        onesb = singles.tile([P, P], BF16)
        zerosb = singles.tile([P, GW], BF16)
        exrowb = singles.tile([P, N_JB * GW], BF16)
        exrow_dram = dram.tile([2, EX_R], BF16)

        nc.vector.memset(onesb, 1.0)
        nc.vector.memset(zerosb, 0.0)

        def emit_ex_quad(jb, q):
            u0 = 4 * q
            t0 = jb * EX_PER_JB + u0
            exq = exl.tile([P, 4, D], F32, tag="exq")
            for h in range(2):
                nc.sync.dma_start(
                    out=exq[:, 2 * h:2 * h + 2, :],
                    in_=ex_d[(t0 + 2 * h) * P:(t0 + 2 * h + 2) * P, :]
                    .rearrange("(t p) d -> p t d", p=P))
            # exsq columns via ACT square with fused accumulate (per tile)
            for u in range(4):
                nc.scalar.activation(
                    out=scratch.tile([P, D], BF16, tag="sqo", name="sqo"),
                    in_=exq[:, u, :], func=AF.Square,
                    accum_out=exsq_cols[jb][:, u0 + u:u0 + u + 1])
            # whole-quad bf16 cast (DVE 2x_2P), XBAR transpose, fp8 casts (DVE)
            exb = casts.tile([P, 4, D], BF16, tag="exb")
            nc.vector.tensor_copy(out=exb, in_=exq)
            tb = tbuf.tile([P, 4 * KC, P], BF16, tag="tb")
            nc.sync.dma_start_transpose(tb, exb)
            for h in range(2):
                dst = exT8s[jb][:, :, (u0 + 2 * h) * P:(u0 + 2 * h + 2) * P]
                nc.vector.tensor_copy(
                    out=dst.rearrange("p k (t f) -> p k t f", t=2),
                    in_=tb[:, 8 * h:8 * h + 8, :].rearrange("p (t k) f -> p k t f", t=2))

        def emit_exrow(jb):
            # hi/lo bf16 split of exsq -> rows 0/1 via DRAM scatter; all these
            # DMAs ride the GpSimd DGE so the SP load queue never blocks.
            cols = exsq_cols[jb]
            hi = scratch.tile([P, EX_PER_JB], BF16, tag="hi")
            lo = scratch.tile([P, EX_PER_JB], BF16, tag="lo")
            nc.vector.tensor_copy(out=hi, in_=cols)
            nc.vector.tensor_tensor(out=lo, in0=cols, in1=hi, op=ALU.subtract)
            sl = slice(jb * GW, (jb + 1) * GW)
            nc.gpsimd.dma_start(
                out=exrow_dram[0:1, sl].rearrange("o (u p) -> (o p) u", p=P), in_=hi)
            nc.gpsimd.dma_start(
                out=exrow_dram[1:2, sl].rearrange("o (u p) -> (o p) u", p=P), in_=lo)
            nc.gpsimd.dma_start(out=exrowb[0:2, sl], in_=exrow_dram[:, sl])
            for r in (32, 64, 96):
                nc.gpsimd.dma_start(out=exrowb[r:r + 2, sl], in_=exrowb[0:2, sl])

        def emit_im_pair(k):
            t0 = 2 * k
            imp = iml.tile([P, 2, D], F32, tag="imp")
            sp = iml.tile([P, 2, D], F32, tag="sp")
            nc.sync.dma_start(
                out=imp,
                in_=im_d[t0 * P:(t0 + 2) * P, :].rearrange("(t p) d -> p t d", p=P))
            nc.sync.dma_start(
                out=sp,
                in_=s_d[t0 * P:(t0 + 2) * P, :].rearrange("(t p) d -> p t d", p=P))
            imb = casts.tile([P, 2, D], BF16, tag="imb")
            nc.vector.tensor_scalar_mul(imb, imp, -2.0)
            for u in range(2):
                t = t0 + u
                st1 = scratch.tile([P, 6], F32, tag="st1")
                nc.vector.bn_stats(out=st1, in_=imp[:, u, :])
                nc.vector.bn_aggr(out=mv_im[:, t, :], in_=st1)
                diff = scratch.tile([P, D], F32, tag="diff")
                nc.gpsimd.tensor_tensor(out=diff, in0=imp[:, u, :], in1=sp[:, u, :],
                                        op=ALU.subtract)
                st2 = scratch.tile([P, 6], F32, tag="st2")
                nc.vector.bn_stats(out=st2, in_=diff)
                nc.vector.bn_aggr(out=mv_dd[:, t, :], in_=st2)
            tb = tbuf.tile([P, 2 * KC, P], BF16, tag="tbi")
            nc.sync.dma_start_transpose(tb, imb)
            dst = imT8[:, :, t0 * P:(t0 + 2) * P]
            nc.vector.tensor_copy(
                out=dst.rearrange("p k (t f) -> p k t f", t=2),
                in_=tb.rearrange("p (t k) f -> p k t f", t=2))

        def emit_fin(b4):
            # batched finalize for tiles [4b, 4b+4): imsq, dd, cc
            sl = slice(4 * b4, 4 * b4 + 4)
            for mv, dest in ((mv_im, imsq), (mv_dd, None)):
                t4 = scratch.tile([P, 4], F32, tag="t4", name="t4")
                nc.vector.tensor_tensor(out=t4, in0=mv[:, sl, 0], in1=mv[:, sl, 0],
                                        op=ALU.mult)
                nc.vector.tensor_tensor(out=t4, in0=t4, in1=mv[:, sl, 1], op=ALU.add)
                if dest is not None:
                    nc.vector.tensor_scalar_mul(dest[:, sl], t4, float(D))
                else:
                    nc.vector.tensor_scalar_mul(t4, t4, float(D))
                    ccs = scratch.tile([P, 4], F32, tag="ccs")
                    nc.scalar.activation(out=ccs, in_=t4, func=AF.Sqrt)
                    nc.vector.tensor_scalar_add(cc[:, sl], ccs, MARGIN)

        # ---- preamble ----
        emit_ex_quad(0, 0)
        emit_ex_quad(0, 1)
        emit_im_pair(0)
        emit_ex_quad(0, 2)
        emit_im_pair(1)
        emit_ex_quad(0, 3)
        emit_exrow(0)
        emit_fin(0)
        emit_im_pair(2)
        emit_im_pair(3)
        emit_fin(1)
        emit_im_pair(4)
        emit_im_pair(5)
        emit_fin(2)
        emit_ex_quad(1, 0)
        emit_im_pair(6)
        emit_ex_quad(1, 1)
        emit_im_pair(7)
        emit_fin(3)
        emit_ex_quad(1, 2)
        emit_ex_quad(1, 3)
        emit_exrow(1)
        nc.sync.dma_start(out=cvec_d, in_=cc)

        # ---- main loop ----
        for jb in range(N_JB):
            for it in range(N_IT):
                g = jb * N_IT + it
                ps = psum.tile([P, GW], F32, tag="mm")

                def mm_exsq(start, stop):
                    for b in range(BANKS):
                        r = 32 * b
                        nc.tensor.matmul(
                            ps[:, b * NJ:(b + 1) * NJ],
                            onesb[r:r + 2, :],
                            exrowb[r:r + 2, jb * GW + b * NJ:jb * GW + (b + 1) * NJ],
                            start=start, stop=stop, tile_position=(r, 0))

                def mm_dr(c, start, stop):
                    for b in range(BANKS):
                        nc.tensor.matmul(
                            ps[:, b * NJ:(b + 1) * NJ],
                            imT8[:, 2 * c:2 * c + 2, it * P:(it + 1) * P],
                            exT8s[jb][:, 2 * c:2 * c + 2, b * NJ:(b + 1) * NJ],
                            start=start, stop=stop, perf_mode=DR)

                if g < 2:
                    mm_dr(0, True, False)
                    mm_dr(1, False, False)
                    mm_exsq(False, True)
                else:
                    mm_exsq(True, False)
                    mm_dr(0, False, False)
                    mm_dr(1, False, True)

                sq = sqp.tile([P, GW], BF16, tag="sq")
                nc.scalar.activation(out=sq, in_=ps, func=AF.Sqrt,
                                     bias=imsq[:, it:it + 1], scale=1.0)
                kind = GROUP_KIND[g]
                mout = sqp.tile([P, GW], BF16, tag="mout")
                if kind == 'act':
                    nc.scalar.activation(
                        out=mout, in_=sq, func=AF.Relu,
                        bias=cc[:, it:it + 1], scale=-1.0,
                        accum_out=acc_sb[:, g:g + 1])
                elif kind == 'stt':
                    nc.vector.scalar_tensor_tensor(
                        out=mout, in0=sq, scalar=cc[:, it:it + 1], in1=zerosb,
                        op0=ALU.min, op1=ALU.add,
                        accum_out=acc_sb[:, g:g + 1])
                else:
                    nc.vector.tensor_scalar(mout, sq, cc[:, it:it + 1], 0.0,
                                            ALU.min, ALU.add,
                                            accum_out=acc_sb[:, g:g + 1])

        nc.sync.dma_start(out=acc_d, in_=acc_sb)


def build_program():
    nc = bacc.Bacc("TRN2", target_bir_lowering=False, debug=False)
    im_d = nc.dram_tensor("im", [IM_R, D], F32, kind="ExternalInput").ap()
    s_d = nc.dram_tensor("s", [IM_R, D], F32, kind="ExternalInput").ap()
    ex_d = nc.dram_tensor("ex", [EX_R, D], F32, kind="ExternalInput").ap()
    acc_d = nc.dram_tensor("acc", [P, N_G], F32, kind="ExternalOutput").ap()
    cvec_d = nc.dram_tensor("cvec", [P, N_IT], F32, kind="ExternalOutput").ap()
    with tile.TileContext(nc) as tc:
        _emit(tc, nc, im_d, s_d, ex_d, acc_d, cvec_d)
    nc.compile()
    return nc


def get_program():
    if "nc" not in _CACHE:
        _CACHE["nc"] = build_program()
    return _CACHE["nc"]


def make_in_maps(im, s, ex_s):
    in_maps = []
    for c in range(8):
        ig, jg = divmod(c, J_GROUPS)
        in_maps.append({
            "im": np.ascontiguousarray(im[ig * IM_R:(ig + 1) * IM_R], dtype=np.float32),
            "s": np.ascontiguousarray(s[ig * IM_R:(ig + 1) * IM_R], dtype=np.float32),
            "ex": np.ascontiguousarray(ex_s[jg * EX_R:(jg + 1) * EX_R], dtype=np.float32),
        })
    return in_maps


def finish(results):
    total = 0.0
    for r in results:
        cvec = np.asarray(r["cvec"], dtype=np.float64)
        acc = np.asarray(r["acc"], dtype=np.float64)
        csum = cvec.sum(axis=0)
        for g in range(N_G):
            it = g % N_IT
            if GROUP_KIND[g] == 'act':
                total += acc[:, g].sum()
            else:
                total += GW * csum[it] - acc[:, g].sum()
    return np.array(total / (float(N) * float(N)), dtype=np.float32)


def kernel(im, s, ex_s):
    nc = get_program()
    res = bass_utils.run_bass_kernel_spmd(nc, make_in_maps(im, s, ex_s),
                                          core_ids=list(range(8)))
    return finish(res.results)


if __name__ == "__main__":
    rng = np.random.default_rng(0)
    im = rng.standard_normal((N, D), dtype=np.float32)
    s = rng.standard_normal((N, D), dtype=np.float32)
    ex = rng.standard_normal((N, D), dtype=np.float32)
    print(kernel(im, s, ex))


# revision 15
# speedup vs baseline: 1.1665x; 1.1665x over previous
"""Trainium2 Bass kernel: CapOnlyContrastiveLoss (margin contrastive loss, mean reduction).

reference math (N=8192, D=512, margin=0.2):
    scores[i,j]  = -||im_i - ex_j||        (via gemm identity)
    diag[i]      = -||im_i - s_i||         (only diag of l2_sim(im, s) is used)
    loss         = mean(relu(margin + scores - diag))

Strategy (v4): 4x2 core grid, fp8 DoubleRow MMs + row-tiled bf16 exsq-fold MMs,
grouped 4-bank epilogue.  vs v3: exsq scatter DMAs issued from the GpSimd DGE
(no SP head-of-line blocking), whole-quad DVE bf16 casts, fp8 casts on DVE,
im/dd stats via DVE bn_stats, and a 3-way epilogue split (DVE STT / DVE
tensor_scalar+accum / ACT relu+accum) to compare engine costs on HW.
"""

import numpy as np

import concourse.bacc as bacc
import concourse.bass as bass
import concourse.tile as tile
from concourse import bass_utils, mybir

N, D = 8192, 512
MARGIN = 0.2
P = 128
NJ = 512
GW = 2048
I_GROUPS, J_GROUPS = 4, 2
IM_R = N // I_GROUPS  # 2048
EX_R = N // J_GROUPS  # 4096
KC = D // P  # 4
N_IT = IM_R // P  # 16
N_JB = EX_R // GW  # 2
BANKS = GW // NJ  # 4
EX_PER_JB = GW // P  # 16
N_G = N_JB * N_IT  # 32

# per-group epilogue kind: 'stt' (DVE scalar_tensor_tensor min+acc),
# 'cache' (DVE tensor_scalar min+acc), 'act' (ACT relu(c-sq)+acc)
GROUP_KIND = ['stt'] * 16 + ['cache'] * 12 + ['act'] * 4

F32 = mybir.dt.float32
BF16 = mybir.dt.bfloat16
FP8 = mybir.dt.float8e4
AF = mybir.ActivationFunctionType
ALU = mybir.AluOpType
DR = mybir.MatmulPerfMode.DoubleRow

_CACHE = {}


def _emit(tc, nc, im_d, s_d, ex_d, acc_d, cvec_d):
    from contextlib import ExitStack

    with ExitStack() as ctx:
        singles = ctx.enter_context(tc.tile_pool(name="singles", bufs=1))
        exl = ctx.enter_context(tc.tile_pool(name="exl", bufs=3))
        iml = ctx.enter_context(tc.tile_pool(name="iml", bufs=3))
        casts = ctx.enter_context(tc.tile_pool(name="casts", bufs=3))
        tbuf = ctx.enter_context(tc.tile_pool(name="tbuf", bufs=3))
        scratch = ctx.enter_context(tc.tile_pool(name="scratch", bufs=6))
        sqp = ctx.enter_context(tc.tile_pool(name="sqp", bufs=2))
        psum = ctx.enter_context(tc.tile_pool(name="psum", bufs=2, space="PSUM"))
        dram = ctx.enter_context(tc.tile_pool(name="dram", bufs=1, space="DRAM"))

        imT8 = singles.tile([P, KC, IM_R], FP8)
        exT8s = [singles.tile([P, KC, GW], FP8, name=f"exT8_{j}") for j in range(N_JB)]
        imsq = singles.tile([P, N_IT], F32)
        cc = singles.tile([P, N_IT], F32)
        mv_im = singles.tile([P, N_IT, 2], F32)
        mv_dd = singles.tile([P, N_IT, 2], F32)
        exsq_cols = [singles.tile([P, EX_PER_JB], F32, name=f"exsqc{j}")
                     for j in range(N_JB)]
        acc_sb = singles.tile([P, N_G], F32)
        onesb = singles.tile([P, P], BF16)
        zerosb = singles.tile([P, GW], BF16)
        exrowb = singles.tile([P, N_JB * GW], BF16)
        exrow_dram = dram.tile([2, EX_R], BF16)

        nc.vector.memset(onesb, 1.0)
        nc.vector.memset(zerosb, 0.0)

        def emit_ex_quad(jb, q):
            u0 = 4 * q
            t0 = jb * EX_PER_JB + u0
            exq = exl.tile([P, 4, D], F32, tag="exq")
            nc.sync.dma_start(
                out=exq,
                in_=ex_d[t0 * P:(t0 + 4) * P, :].rearrange("(t p) d -> p t d", p=P))
            # exsq columns via ACT square with fused accumulate (per tile)
            for u in range(4):
                nc.scalar.activation(
                    out=scratch.tile([P, D], BF16, tag="sqo", name="sqo"),
                    in_=exq[:, u, :], func=AF.Square,
                    accum_out=exsq_cols[jb][:, u0 + u:u0 + u + 1])
            # whole-quad bf16 cast (DVE 2x_2P), XBAR transpose, fp8 casts (DVE)
            exb = casts.tile([P, 4, D], BF16, tag="exb")
            nc.vector.tensor_copy(out=exb, in_=exq)
            tb = tbuf.tile([P, 4 * KC, P], BF16, tag="tb")
            nc.sync.dma_start_transpose(tb, exb)
            for h in range(2):
                dst = exT8s[jb][:, :, (u0 + 2 * h) * P:(u0 + 2 * h + 2) * P]
                nc.vector.tensor_copy(
                    out=dst.rearrange("p k (t f) -> p k t f", t=2),
                    in_=tb[:, 8 * h:8 * h + 8, :].rearrange("p (t k) f -> p k t f", t=2))

        def emit_exrow(jb):
            # hi/lo bf16 split of exsq -> rows 0/1 via DRAM scatter; all these
            # DMAs ride the GpSimd DGE so the SP load queue never blocks.
            cols = exsq_cols[jb]
            hi = scratch.tile([P, EX_PER_JB], BF16, tag="hi")
            lo = scratch.tile([P, EX_PER_JB], BF16, tag="lo")
            nc.vector.tensor_copy(out=hi, in_=cols)
            nc.vector.tensor_tensor(out=lo, in0=cols, in1=hi, op=ALU.subtract)
            sl = slice(jb * GW, (jb + 1) * GW)
            nc.gpsimd.dma_start(
                out=exrow_dram[0:1, sl].rearrange("o (u p) -> (o p) u", p=P), in_=hi)
            nc.gpsimd.dma_start(
                out=exrow_dram[1:2, sl].rearrange("o (u p) -> (o p) u", p=P), in_=lo)
            nc.gpsimd.dma_start(out=exrowb[0:2, sl], in_=exrow_dram[:, sl])
            for r in (32, 64, 96):
                nc.gpsimd.dma_start(out=exrowb[r:r + 2, sl], in_=exrowb[0:2, sl])

        def emit_im_pair(k):
            t0 = 2 * k
            imp = iml.tile([P, 2, D], F32, tag="imp")
            sp = iml.tile([P, 2, D], F32, tag="sp")
            nc.sync.dma_start(
                out=imp,
                in_=im_d[t0 * P:(t0 + 2) * P, :].rearrange("(t p) d -> p t d", p=P))
            nc.sync.dma_start(
                out=sp,
                in_=s_d[t0 * P:(t0 + 2) * P, :].rearrange("(t p) d -> p t d", p=P))
            imb = casts.tile([P, 2, D], BF16, tag="imb")
            nc.vector.tensor_scalar_mul(imb, imp, -2.0)
            for u in range(2):
                t = t0 + u
                st1 = scratch.tile([P, 6], F32, tag="st1")
                nc.vector.bn_stats(out=st1, in_=imp[:, u, :])
                nc.vector.bn_aggr(out=mv_im[:, t, :], in_=st1)
                diff = scratch.tile([P, D], F32, tag="diff")
                nc.gpsimd.tensor_tensor(out=diff, in0=imp[:, u, :], in1=sp[:, u, :],
                                        op=ALU.subtract)
                st2 = scratch.tile([P, 6], F32, tag="st2")
                nc.vector.bn_stats(out=st2, in_=diff)
                nc.vector.bn_aggr(out=mv_dd[:, t, :], in_=st2)
            tb = tbuf.tile([P, 2 * KC, P], BF16, tag="tbi")
            nc.sync.dma_start_transpose(tb, imb)
            dst = imT8[:, :, t0 * P:(t0 + 2) * P]
            nc.vector.tensor_copy(
                out=dst.rearrange("p k (t f) -> p k t f", t=2),
                in_=tb.rearrange("p (t k) f -> p k t f", t=2))

        def emit_fin(b4):
            # batched finalize for tiles [4b, 4b+4): imsq, dd, cc
            sl = slice(4 * b4, 4 * b4 + 4)
            for mv, dest in ((mv_im, imsq), (mv_dd, None)):
                t4 = scratch.tile([P, 4], F32, tag="t4", name="t4")
                nc.vector.tensor_tensor(out=t4, in0=mv[:, sl, 0], in1=mv[:, sl, 0],
                                        op=ALU.mult)
                nc.vector.tensor_tensor(out=t4, in0=t4, in1=mv[:, sl, 1], op=ALU.add)
                if dest is not None:
                    nc.vector.tensor_scalar_mul(dest[:, sl], t4, float(D))
                else:
                    nc.vector.tensor_scalar_mul(t4, t4, float(D))
                    ccs = scratch.tile([P, 4], F32, tag="ccs")
                    nc.scalar.activation(out=ccs, in_=t4, func=AF.Sqrt)
                    nc.vector.tensor_scalar_add(cc[:, sl], ccs, MARGIN)

        # ---- preamble ----
        emit_ex_quad(0, 0)
        emit_ex_quad(0, 1)
        emit_im_pair(0)
        emit_ex_quad(0, 2)
        emit_im_pair(1)
        emit_ex_quad(0, 3)
        emit_exrow(0)
        emit_fin(0)
        emit_im_pair(2)
        emit_im_pair(3)
        emit_fin(1)
        emit_im_pair(4)
        emit_im_pair(5)
        emit_fin(2)
        emit_ex_quad(1, 0)
        emit_im_pair(6)
        emit_ex_quad(1, 1)
        emit_im_pair(7)
        emit_fin(3)
        emit_ex_quad(1, 2)
        emit_ex_quad(1, 3)
        emit_exrow(1)
        nc.sync.dma_start(out=cvec_d, in_=cc)

        # ---- main loop ----
        for jb in range(N_JB):
            for it in range(N_IT):
                g = jb * N_IT + it
                ps = psum.tile([P, GW], F32, tag="mm")

                def mm_exsq(start, stop):
                    for b in range(BANKS):
                        r = 32 * b
                        nc.tensor.matmul(
                            ps[:, b * NJ:(b + 1) * NJ],
                            onesb[r:r + 2, :],
                            exrowb[r:r + 2, jb * GW + b * NJ:jb * GW + (b + 1) * NJ],
                            start=start, stop=stop, tile_position=(r, 0))

                def mm_dr(c, start, stop):
                    for b in range(BANKS):
                        nc.tensor.matmul(
                            ps[:, b * NJ:(b + 1) * NJ],
                            imT8[:, 2 * c:2 * c + 2, it * P:(it + 1) * P],
                            exT8s[jb][:, 2 * c:2 * c + 2, b * NJ:(b + 1) * NJ],
                            start=start, stop=stop, perf_mode=DR)

                if g < 2:
                    mm_dr(0, True, False)
                    mm_dr(1, False, False)
                    mm_exsq(False, True)
                else:
                    mm_exsq(True, False)
                    mm_dr(0, False, False)
                    mm_dr(1, False, True)

                sq = sqp.tile([P, GW], BF16, tag="sq")
                nc.scalar.activation(out=sq, in_=ps, func=AF.Sqrt,
                                     bias=imsq[:, it:it + 1], scale=1.0)
                kind = GROUP_KIND[g]
                mout = sqp.tile([P, GW], BF16, tag="mout")
                if kind == 'act':
                    nc.scalar.activation(
                        out=mout, in_=sq, func=AF.Relu,
                        bias=cc[:, it:it + 1], scale=-1.0,
                        accum_out=acc_sb[:, g:g + 1])
                elif kind == 'stt':
                    nc.vector.scalar_tensor_tensor(
                        out=mout, in0=sq, scalar=cc[:, it:it + 1], in1=zerosb,
                        op0=ALU.min, op1=ALU.add,
                        accum_out=acc_sb[:, g:g + 1])
                else:
                    nc.vector.tensor_scalar(mout, sq, cc[:, it:it + 1], 0.0,
                                            ALU.min, ALU.add,
                                            accum_out=acc_sb[:, g:g + 1])

        nc.sync.dma_start(out=acc_d, in_=acc_sb)


def build_program():
    nc = bacc.Bacc("TRN2", target_bir_lowering=False, debug=False)
    im_d = nc.dram_tensor("im", [IM_R, D], F32, kind="ExternalInput").ap()
    s_d = nc.dram_tensor("s", [IM_R, D], F32, kind="ExternalInput").ap()
    ex_d = nc.dram_tensor("ex", [EX_R, D], F32, kind="ExternalInput").ap()
    acc_d = nc.dram_tensor("acc", [P, N_G], F32, kind="ExternalOutput").ap()
    cvec_d = nc.dram_tensor("cvec", [P, N_IT], F32, kind="ExternalOutput").ap()
    with tile.TileContext(nc) as tc:
        _emit(tc, nc, im_d, s_d, ex_d, acc_d, cvec_d)
    nc.compile()
    return nc


def get_program():
    if "nc" not in _CACHE:
        _CACHE["nc"] = build_program()
    return _CACHE["nc"]


def make_in_maps(im, s, ex_s):
    in_maps = []
    for c in range(8):
        ig, jg = divmod(c, J_GROUPS)
        in_maps.append({
            "im": np.ascontiguousarray(im[ig * IM_R:(ig + 1) * IM_R], dtype=np.float32),
            "s": np.ascontiguousarray(s[ig * IM_R:(ig + 1) * IM_R], dtype=np.float32),
            "ex": np.ascontiguousarray(ex_s[jg * EX_R:(jg + 1) * EX_R], dtype=np.float32),
        })
    return in_maps


def finish(results):
    total = 0.0
    for r in results:
        cvec = np.asarray(r["cvec"], dtype=np.float64)
        acc = np.asarray(r["acc"], dtype=np.float64)
        csum = cvec.sum(axis=0)
        for g in range(N_G):
            it = g % N_IT
            if GROUP_KIND[g] == 'act':
                total += acc[:, g].sum()
            else:
                total += GW * csum[it] - acc[:, g].sum()
    return np.array(total / (float(N) * float(N)), dtype=np.float32)


def kernel(im, s, ex_s):
    nc = get_program()
    res = bass_utils.run_bass_kernel_spmd(nc, make_in_maps(im, s, ex_s),
                                          core_ids=list(range(8)))
    return finish(res.results)


if __name__ == "__main__":
    rng = np.random.default_rng(0)
    im = rng.standard_normal((N, D), dtype=np.float32)
    s = rng.standard_normal((N, D), dtype=np.float32)
    ex = rng.standard_normal((N, D), dtype=np.float32)
    print(kernel(im, s, ex))
